# revision 51
# baseline (speedup 1.0000x reference)
"""Trainium2 Bass kernel for nn_ActionHead (ragged cross-attention pooling).

Math (per sample b, per head h):
    k      = feat_b @ Wk_h                         (n, D)
    vsum_n = (feat_b @ Wv_h).sum(-1) = feat_b @ rowsum(Wv_h)       (n,)
    s      = k @ q_h.T / sqrt(D) = feat_b @ (Wk_h @ q_h.T) / 16    (n, Q_h)
    out    = sum_n exp(s_nq) * vsum_n / sum_n exp(s_nq)            (Q_h,)

Sharding: data-parallel over the 8 samples (one sample per NeuronCore),
query banks + projection weights replicated; host gathers the (B, Q)
outputs.  With this data distribution |s| << 1 (verified by a cheap host
bound), so the softmax max-subtraction is unnecessary and exp() stays in
a benign range.

Device pipeline per core (option "scores.T"-layout):
    - DMA-cast fp32->bf16 in DRAM, xbar DMA-transpose to SBUF:
        featT (128,2,4096), qT (128,2,3456), wkT per head.
    - W_eff = Wk @ qcat.T via PE, kept bf16 as the stationary matmul operand.
    - vrep_h (128, 4096) fp16: per-head vsum row replicated across
      partitions, built with PE outer-product matmuls.
    - Main loop over 27 query tiles x 2 point-halves:
        PE:  scores.T tile (128q, 2048n) = W_eff_tile.T @ featT  (bf16, fp32 acc)
        ACT: e = exp(s/16) PSUM->SBUF fp16, accum_out -> denom column
        DVE/GPSIMD: numer column via (e * vrep) free-axis reduction
    - Epilogue: combine halves, out = numer * recip(denom), DMA (128,27).
"""

import sys

import numpy as np

sys.path.insert(0, "/opt/trn_rl_repo")

import concourse.bacc as bacc
import concourse.bass as bass
import concourse.tile as tile
from concourse import masks, mybir
from concourse.bass_utils import run_bass_kernel_spmd

F32 = mybir.dt.float32
BF16 = mybir.dt.bfloat16
F16 = mybir.dt.float16
AX = mybir.AxisListType
OP = mybir.AluOpType
ACTF = mybir.ActivationFunctionType
ts = bass.ts

B, PTS, D = 8, 4096, 256
QT, QR, QO = 3000, 216, 1
# Queries padded into 26 tiles of 128 rows:
# t: tiles 0..23 (rows 0..2999), r: rows 3072..3287 (tile 24 pure, tile 25
# rows 0..87), o: row 3288 (= tile 25 row 88; tile 25 uses a mixed vsum).
QPAD = 3328
N_QTILES = QPAD // 128
R_OFF, O_OFF = 3072, 3288
NHALF = 2048
SCALE = 1.0 / 16.0  # 1/sqrt(D)

_CACHE = {}


def _build_nc():
    nc = bacc.Bacc(
        "TRN2",
        target_bir_lowering=False,
        debug=False,
        enable_asserts=False,
        num_devices=8,
    )
    feat = nc.dram_tensor("feat", [PTS, D], F32, kind="ExternalInput")[:]
    qcat = nc.dram_tensor("qcat", [QPAD, D], F32, kind="ExternalInput")[:]
    wk = nc.dram_tensor("wk", [3, D, D], F32, kind="ExternalInput")[:]
    wv = nc.dram_tensor("wv", [3, D, D], F32, kind="ExternalInput")[:]
    res = nc.dram_tensor("res", [128, N_QTILES], F32, kind="ExternalOutput")[:]

    with tile.TileContext(nc) as tc:
        _kernel_body(tc, feat, qcat, wk, wv, res)
    nc.compile()
    return nc


def _kernel_body(tc, feat, qcat, wk, wv, res):
    nc = tc.nc

    with tc.tile_pool(name="persist", bufs=1) as persist:
        featT = persist.tile([128, 2, PTS], BF16)
        qT = persist.tile([128, 2, QPAD], BF16)
        wkT = persist.tile([128, 3, 2, D], BF16)
        weff = persist.tile([128, 2, QPAD], BF16)
        vrep = persist.tile([128, 3, PTS], F16)
        wvbar = persist.tile([128, 2, 3], F32)
        ones_bf = persist.tile([128, 128], BF16)
        wvrep = persist.tile([128, 2, 4, 128], BF16)
        identity = persist.tile([128, 128], BF16)
        masks.make_identity(nc, identity[:])
        nc.vector.memset(ones_bf[:], 1.0)

        # ---- load fp32 inputs, transpose via PE into bf16 SBUF layouts, and
        # ---- run the prologue matmuls (W_eff, vrep) as operands become ready.
        with tc.tile_pool(name="ldp", bufs=1) as ldp, \
             tc.tile_pool(name="ldst", bufs=6) as ldst, \
             tc.tile_pool(name="ppsum", bufs=2, space="PSUM") as ppsum, \
             tc.tile_pool(name="tpsum", bufs=3, space="PSUM") as tpsum:

            # ---- all input DMAs, emitted in order of need (HBM-bw bound):
            # wk, qcat chunk0, feat chunks, qcat chunk1, wv last.
            twk = ldp.tile([128, 6, D], F32)
            nc.sync.dma_start(
                twk[:],
                wk.rearrange("h a d -> (h a) d").rearrange("(g p) d -> p g d", p=128),
            )
            qcat_r = qcat.rearrange("(g p) d -> p g d", p=128)
            QB = [0, 8, QPAD // 128]
            q_ld = []
            t = ldp.tile([128, QB[1] - QB[0], D], F32, tag="lq_0", name="t")
            nc.sync.dma_start(t[:], qcat_r[:, QB[0]:QB[1], :])
            q_ld.append((QB[0], QB[1], t))
            feat_r = feat.rearrange("(g p) d -> p g d", p=128)
            FB = [0, 8, 16, 24, 32]
            f_ld = []
            for ci, (g0, g1) in enumerate(zip(FB[:-1], FB[1:])):
                t = ldp.tile([128, g1 - g0, D], F32, tag=f"lf_{ci}", name="t")
                nc.sync.dma_start(t[:], feat_r[:, g0:g1, :])
                f_ld.append((g0, g1, t))
            t = ldp.tile([128, QB[2] - QB[1], D], F32, tag="lq_1", name="t")
            nc.sync.dma_start(t[:], qcat_r[:, QB[1]:QB[2], :])
            q_ld.append((QB[1], QB[2], t))
            wv_ld = []
            for h in range(3):
                for kt in range(2):
                    t = ldst.tile([128, D], F32, tag=f"wvld{2*h+kt}")
                    nc.sync.dma_start(t[:], wv[h, ts(kt, 128), :])
                    wv_ld.append(t)

            # wvbar[din, h] = rowsum(Wv_h); replicated (128,128) blocks.
            for h in range(3):
                for kt in range(2):
                    nc.vector.tensor_reduce(
                        wvbar[:, kt, h : h + 1], wv_ld[2 * h + kt][:],
                        axis=AX.X, op=OP.add,
                    )
            for h in range(3):
                for kt in range(2):
                    nc.vector.tensor_scalar_mul(
                        wvrep[:, kt, h, :], ones_bf[:], wvbar[:, kt, h : h + 1]
                    )
            # mixed block for tile 25: r everywhere, o at row 88
            for kt in range(2):
                nc.vector.tensor_scalar_mul(
                    wvrep[:, kt, 3, :], ones_bf[:], wvbar[:, kt, 1:2]
                )
                nc.vector.tensor_scalar_mul(
                    wvrep[:, kt, 3, 88:89], ones_bf[:, :1], wvbar[:, kt, 2:3]
                )

            cp_i = [0]

            def cp(dst, ps):
                eng = nc.scalar.copy if cp_i[0] % 2 else nc.vector.tensor_copy
                cp_i[0] += 1
                eng(dst, ps)

            def load_T(dst_T, chunks, tag):
                for g0, g1, t in chunks:
                    tb = ldp.tile([128, int(g1 - g0), D], BF16,
                                  tag=f"{tag}b_{g0}", name="tb")
                    cp(tb[:], t[:])
                    for kt in range(2):
                        for b0 in range(int(g0), int(g1), 4):
                            b1 = min(b0 + 4, int(g1))
                            ps = tpsum.tile([128, 512], BF16, tag="tps")
                            for j in range(b0, b1):
                                nc.tensor.transpose(
                                    ps[:, ts(j - b0, 128)],
                                    tb[:, j - int(g0), ts(kt, 128)],
                                    identity[:],
                                )
                            cp(dst_T[:, kt, b0 * 128 : b1 * 128],
                               ps[:, : (b1 - b0) * 128])

            # wk (smallest) first: per-head layout
            twkb = ldp.tile([128, 6, D], BF16)
            cp(twkb[:], twk[:])
            for h in range(3):
                for kt in range(2):
                    ps = tpsum.tile([128, 512], BF16, tag="tps")
                    for g in range(2):
                        nc.tensor.transpose(
                            ps[:, ts(g, 128)],
                            twkb[:, 2 * h + g, ts(kt, 128)],
                            identity[:],
                        )
                    cp(wkT[:, h, kt, :], ps[:, :256])

            def weff_group(q0, w, h):
                for m in range(2):
                    ps = ppsum.tile([128, 1024], F32, tag="pps")
                    for kt in range(2):
                        for c in range(0, w, 512):
                            cw = min(512, w - c)
                            nc.tensor.matmul(
                                ps[:, c : c + cw],
                                wkT[:, h, kt, ts(m, 128)],
                                qT[:, kt, q0 + c : q0 + c + cw],
                                start=(kt == 0),
                                stop=(kt == 1),
                            )
                    cp(weff[:, m, q0 : q0 + w], ps[:, :w])

            def vrep_chunk(c):
                for h, wslot in ((0, 0), (1, 1), (2, 3)):
                    ps = ppsum.tile([128, 1024], F32, tag="pps")
                    for kt in range(2):
                        for cc in range(2):
                            nc.tensor.matmul(
                                ps[:, ts(cc, 512)],
                                wvrep[:, kt, wslot, :],
                                featT[:, kt, c * 1024 + cc * 512 : c * 1024 + (cc + 1) * 512],
                                start=(kt == 0),
                                stop=(kt == 1),
                            )
                    cp(vrep[:, h, ts(c, 1024)], ps[:])

            load_T(qT, q_ld[:1], "q")
            weff_group(0, 1024, 0)
            load_T(featT, f_ld, "f")
            for c in range(2):
                vrep_chunk(c)
            load_T(qT, q_ld[1:], "q")
            weff_group(1024, 1024, 0)
            weff_group(2048, 1024, 0)
            weff_group(R_OFF, 216, 1)
            weff_group(O_OFF, 40, 2)
            for c in range(2, 4):
                vrep_chunk(c)

        # ---- main loop.
        denoms = persist.tile([128, N_QTILES, 2], F32)
        numers = persist.tile([128, N_QTILES], F32)
        with tc.tile_pool(name="mpsum", bufs=2, space="PSUM") as mpsum, \
             tc.tile_pool(name="epool", bufs=4) as epool, \
             tc.tile_pool(name="prodp", bufs=2) as prodp:
            for qt in range(N_QTILES):
                h = 0 if qt < 24 else (1 if qt == 24 else 2)
                e_t = epool.tile([128, PTS], F16, tag="e")
                for half in range(2):
                    ps = mpsum.tile([128, NHALF], F32, tag="mps")
                    for kt in range(2):
                        for c4 in range(NHALF // 512):
                            n0 = half * NHALF + c4 * 512
                            nc.tensor.matmul(
                                ps[:, ts(c4, 512)],
                                weff[:, kt, ts(qt, 128)],
                                featT[:, kt, n0 : n0 + 512],
                                start=(kt == 0),
                                stop=(kt == 1),
                            )
                    nc.scalar.activation(
                        e_t[:, half * NHALF : (half + 1) * NHALF],
                        ps[:],
                        ACTF.Exp,
                        scale=SCALE,
                        accum_out=denoms[:, qt, half : half + 1],
                    )
                prod = prodp.tile([128, PTS], F16, tag="prodv")
                nc.vector.scalar_tensor_tensor(
                    prod[:],
                    e_t[:],
                    1.0,
                    vrep[:, h, :],
                    op0=OP.mult,
                    op1=OP.mult,
                    accum_out=numers[:, qt : qt + 1],
                )

        # ---- epilogue: combine halves, divide, store.
        dsum = persist.tile([128, N_QTILES], F32)
        rcp = persist.tile([128, N_QTILES], F32)
        outt = persist.tile([128, N_QTILES], F32)
        nc.vector.tensor_add(dsum[:], denoms[:, :, 0], denoms[:, :, 1])
        nc.vector.reciprocal(rcp[:], dsum[:])
        nc.vector.tensor_mul(outt[:], numers[:], rcp[:])
        nc.sync.dma_start(res, outt[:])

def _get_nc():
    if "nc" not in _CACHE:
        _CACHE["nc"] = _build_nc()
    return _CACHE["nc"]


def _fast_path_ok(feat, npb, shapes):
    if not all(shapes):
        return False
    if not np.all(npb == PTS):
        return False
    return True


def _score_bound(feat, wks, qs):
    """Cheap upper bound on max |scores| (pre exp). Fast path requires the
    exp outputs to stay well inside fp16 range with no max-subtraction."""
    fn = float(np.sqrt((feat.astype(np.float64) ** 2).sum(1)).max())
    bound = 0.0
    for Wk, q in zip(wks, qs):
        wn = float(np.linalg.norm(Wk.astype(np.float64)))  # ||Wk||_F >= ||Wk||_2
        qn = float(np.sqrt((q.astype(np.float64) ** 2).sum(1)).max())
        bound = max(bound, fn * wn * qn / 16.0)
    return bound


def _fallback(feat, npb, qs, wks, wvs):
    """Exact host computation for inputs outside the device fast path."""
    feat = feat.astype(np.float64)
    ends = np.cumsum(npb)
    starts = ends - npb
    outs = []
    for q, Wk, Wv in zip(qs, wks, wvs):
        q = q.astype(np.float64)
        k = feat @ Wk.astype(np.float64)
        vsum = (feat @ Wv.astype(np.float64)).sum(-1)
        out = np.zeros((B, q.shape[0]), np.float64)
        for b in range(B):
            s, e = int(starts[b]), int(ends[b])
            sc = (k[s:e] @ q.T) / 16.0
            sc -= sc.max(0, keepdims=True)
            ee = np.exp(sc)
            out[b] = (ee * vsum[s:e, None]).sum(0) / ee.sum(0)
        outs.append(out.astype(np.float32))
    return tuple(outs)


def _run(inputs, trace=False):
    feat = np.ascontiguousarray(np.asarray(inputs["feat"], dtype=np.float32))
    npb = np.asarray(inputs["npoints_in_batch"]).astype(np.int64)
    qs = [np.asarray(inputs[f"queries_{n}"], np.float32) for n in "tro"]
    wks = [np.asarray(inputs[f"Wk_{n}"], np.float32) for n in "tro"]
    wvs = [np.asarray(inputs[f"Wv_{n}"], np.float32) for n in "tro"]

    shapes_ok = (
        feat.shape == (B * PTS, D)
        and npb.shape == (B,)
        and qs[0].shape == (QT, D)
        and qs[1].shape == (QR, D)
        and qs[2].shape == (QO, D)
        and all(w.shape == (D, D) for w in wks + wvs)
    )
    if not (shapes_ok and np.all(npb == PTS) and _score_bound(feat, wks, qs) < 9.0):
        return _fallback(feat, npb, qs, wks, wvs), None

    qcat = np.zeros((QPAD, D), np.float32)
    qcat[0:QT] = qs[0]
    qcat[R_OFF : R_OFF + QR] = qs[1]
    qcat[O_OFF : O_OFF + QO] = qs[2]
    assert O_OFF == R_OFF + QR
    wk = np.ascontiguousarray(np.stack(wks))
    wv = np.ascontiguousarray(np.stack(wvs))

    in_maps = [
        {"feat": feat[b * PTS : (b + 1) * PTS], "qcat": qcat, "wk": wk, "wv": wv}
        for b in range(B)
    ]
    r = run_bass_kernel_spmd(_get_nc(), in_maps, core_ids=list(range(8)), trace=trace)
    flat = np.stack([np.asarray(r.results[b]["res"]).T.reshape(-1) for b in range(B)])
    out = (
        flat[:, 0:QT].copy(),
        flat[:, R_OFF : R_OFF + QR].copy(),
        flat[:, O_OFF : O_OFF + QO].copy(),
    )
    return out, r


def kernel(**inputs):
    out, _ = _run(inputs, trace=False)
    return out


def kernel_traced(**inputs):
    """Like kernel() but returns (outputs, BassKernelResults with trace)."""
    return _run(inputs, trace=True)


# revision 52
# speedup vs baseline: 1.1957x; 1.1957x over previous
"""Trainium2 Bass kernel for nn_ActionHead (ragged cross-attention pooling).

Math (per sample b, per head h):
    k      = feat_b @ Wk_h                         (n, D)
    vsum_n = (feat_b @ Wv_h).sum(-1) = feat_b @ rowsum(Wv_h)       (n,)
    s      = k @ q_h.T / sqrt(D) = feat_b @ (Wk_h @ q_h.T) / 16    (n, Q_h)
    out    = sum_n exp(s_nq) * vsum_n / sum_n exp(s_nq)            (Q_h,)

Sharding: data-parallel over the 8 samples (one sample per NeuronCore),
query banks + projection weights replicated; host gathers the (B, Q)
outputs.  With this data distribution |s| << 1 (verified by a cheap host
bound), so the softmax max-subtraction is unnecessary and exp() stays in
a benign range.

Device pipeline per core (option "scores.T"-layout):
    - DMA-cast fp32->bf16 in DRAM, xbar DMA-transpose to SBUF:
        featT (128,2,4096), qT (128,2,3456), wkT per head.
    - W_eff = Wk @ qcat.T via PE, kept bf16 as the stationary matmul operand.
    - vrep_h (128, 4096) fp16: per-head vsum row replicated across
      partitions, built with PE outer-product matmuls.
    - Main loop over 27 query tiles x 2 point-halves:
        PE:  scores.T tile (128q, 2048n) = W_eff_tile.T @ featT  (bf16, fp32 acc)
        ACT: e = exp(s/16) PSUM->SBUF fp16, accum_out -> denom column
        DVE/GPSIMD: numer column via (e * vrep) free-axis reduction
    - Epilogue: combine halves, out = numer * recip(denom), DMA (128,27).
"""

import sys

import numpy as np

sys.path.insert(0, "/opt/trn_rl_repo")

import concourse.bacc as bacc
import concourse.bass as bass
import concourse.tile as tile
from concourse import masks, mybir
from concourse.bass_utils import run_bass_kernel_spmd

F32 = mybir.dt.float32
BF16 = mybir.dt.bfloat16
F16 = mybir.dt.float16
AX = mybir.AxisListType
OP = mybir.AluOpType
ACTF = mybir.ActivationFunctionType
ts = bass.ts

B, PTS, D = 8, 4096, 256
QT, QR, QO = 3000, 216, 1
# Queries padded into 26 tiles of 128 rows:
# t: tiles 0..23 (rows 0..2999), r: rows 3072..3287 (tile 24 pure, tile 25
# rows 0..87), o: row 3288 (= tile 25 row 88; tile 25 uses a mixed vsum).
QPAD = 3328
N_QTILES = QPAD // 128
R_OFF, O_OFF = 3072, 3288
NHALF = 2048
SCALE = 1.0 / 16.0  # 1/sqrt(D)

_CACHE = {}


def _build_nc():
    nc = bacc.Bacc(
        "TRN2",
        target_bir_lowering=False,
        debug=False,
        enable_asserts=False,
        num_devices=8,
    )
    feat = nc.dram_tensor("feat", [PTS, D], F32, kind="ExternalInput")[:]
    qcat = nc.dram_tensor("qcat", [QPAD, D], F32, kind="ExternalInput")[:]
    wk = nc.dram_tensor("wk", [3, D, D], F32, kind="ExternalInput")[:]
    wv = nc.dram_tensor("wv", [3, D, D], F32, kind="ExternalInput")[:]
    res = nc.dram_tensor("res", [128, N_QTILES], F32, kind="ExternalOutput")[:]

    with tile.TileContext(nc) as tc:
        _kernel_body(tc, feat, qcat, wk, wv, res)
    nc.compile()
    return nc


def _kernel_body(tc, feat, qcat, wk, wv, res):
    nc = tc.nc

    with tc.tile_pool(name="persist", bufs=1) as persist:
        featT = persist.tile([128, 2, PTS], BF16)
        qT = persist.tile([128, 2, QPAD], BF16)
        wkT = persist.tile([128, 3, 2, D], BF16)
        weff = persist.tile([128, 2, QPAD], BF16)
        vrep = persist.tile([128, 3, PTS], F16)
        wvbar = persist.tile([128, 2, 3], F32)
        ones_bf = persist.tile([128, 128], BF16)
        wvrep = persist.tile([128, 2, 4, 128], BF16)
        identity = persist.tile([128, 128], F32)
        masks.make_identity(nc, identity[:])
        nc.vector.memset(ones_bf[:], 1.0)

        # ---- load fp32 inputs, transpose via PE into bf16 SBUF layouts, and
        # ---- run the prologue matmuls (W_eff, vrep) as operands become ready.
        with tc.tile_pool(name="ldp", bufs=1) as ldp, \
             tc.tile_pool(name="ldst", bufs=6) as ldst, \
             tc.tile_pool(name="ppsum", bufs=2, space="PSUM") as ppsum, \
             tc.tile_pool(name="tpsum", bufs=3, space="PSUM") as tpsum:

            # ---- all input DMAs, emitted in order of need (HBM-bw bound):
            # wk, qcat chunk0, feat chunks, qcat chunk1, wv last.
            twk = ldp.tile([128, 6, D], F32)
            nc.sync.dma_start(
                twk[:],
                wk.rearrange("h a d -> (h a) d").rearrange("(g p) d -> p g d", p=128),
            )
            qcat_r = qcat.rearrange("(g p) d -> p g d", p=128)
            QB = [0, 8, QPAD // 128]
            q_ld = []
            t = ldp.tile([128, QB[1] - QB[0], D], F32, tag="lq_0", name="t")
            nc.sync.dma_start(t[:], qcat_r[:, QB[0]:QB[1], :])
            q_ld.append((QB[0], QB[1], t))
            feat_r = feat.rearrange("(g p) d -> p g d", p=128)
            FB = [0, 8, 16, 24, 32]
            f_ld = []
            for ci, (g0, g1) in enumerate(zip(FB[:-1], FB[1:])):
                t = ldp.tile([128, g1 - g0, D], F32, tag=f"lf_{ci}", name="t")
                nc.sync.dma_start(t[:], feat_r[:, g0:g1, :])
                f_ld.append((g0, g1, t))
            t = ldp.tile([128, QB[2] - QB[1], D], F32, tag="lq_1", name="t")
            nc.sync.dma_start(t[:], qcat_r[:, QB[1]:QB[2], :])
            q_ld.append((QB[1], QB[2], t))
            wv_ld = []
            for h in range(3):
                for kt in range(2):
                    t = ldst.tile([128, D], F32, tag=f"wvld{2*h+kt}")
                    nc.sync.dma_start(t[:], wv[h, ts(kt, 128), :])
                    wv_ld.append(t)

            # wvbar[din, h] = rowsum(Wv_h); replicated (128,128) blocks.
            for h in range(3):
                for kt in range(2):
                    nc.vector.tensor_reduce(
                        wvbar[:, kt, h : h + 1], wv_ld[2 * h + kt][:],
                        axis=AX.X, op=OP.add,
                    )
            for h in range(3):
                for kt in range(2):
                    nc.vector.tensor_scalar_mul(
                        wvrep[:, kt, h, :], ones_bf[:], wvbar[:, kt, h : h + 1]
                    )
            # mixed block for tile 25: r everywhere, o at row 88
            for kt in range(2):
                nc.vector.tensor_scalar_mul(
                    wvrep[:, kt, 3, :], ones_bf[:], wvbar[:, kt, 1:2]
                )
                nc.vector.tensor_scalar_mul(
                    wvrep[:, kt, 3, 88:89], ones_bf[:, :1], wvbar[:, kt, 2:3]
                )

            cp_i = [0]

            def cp(dst, ps):
                eng = nc.scalar.copy if cp_i[0] % 2 else nc.vector.tensor_copy
                cp_i[0] += 1
                eng(dst, ps)

            def load_T(dst_T, chunks):
                for g0, g1, t in chunks:
                    for kt in range(2):
                        for b0 in range(int(g0), int(g1), 4):
                            b1 = min(b0 + 4, int(g1))
                            ps = tpsum.tile([128, 512], F32, tag="tps")
                            for j in range(b0, b1):
                                nc.tensor.transpose(
                                    ps[:, ts(j - b0, 128)],
                                    t[:, j - int(g0), ts(kt, 128)],
                                    identity[:],
                                )
                            cp(dst_T[:, kt, b0 * 128 : b1 * 128],
                               ps[:, : (b1 - b0) * 128])

            # wk (smallest) first: per-head layout
            for h in range(3):
                for kt in range(2):
                    ps = tpsum.tile([128, 512], F32, tag="tps")
                    for g in range(2):
                        nc.tensor.transpose(
                            ps[:, ts(g, 128)],
                            twk[:, 2 * h + g, ts(kt, 128)],
                            identity[:],
                        )
                    cp(wkT[:, h, kt, :], ps[:, :256])

            def weff_group(q0, w, h):
                for m in range(2):
                    ps = ppsum.tile([128, 1024], F32, tag="pps")
                    for kt in range(2):
                        for c in range(0, w, 512):
                            cw = min(512, w - c)
                            nc.tensor.matmul(
                                ps[:, c : c + cw],
                                wkT[:, h, kt, ts(m, 128)],
                                qT[:, kt, q0 + c : q0 + c + cw],
                                start=(kt == 0),
                                stop=(kt == 1),
                            )
                    cp(weff[:, m, q0 : q0 + w], ps[:, :w])

            def vrep_chunk(c):
                for h, wslot in ((0, 0), (1, 1), (2, 3)):
                    ps = ppsum.tile([128, 1024], F32, tag="pps")
                    for kt in range(2):
                        for cc in range(2):
                            nc.tensor.matmul(
                                ps[:, ts(cc, 512)],
                                wvrep[:, kt, wslot, :],
                                featT[:, kt, c * 1024 + cc * 512 : c * 1024 + (cc + 1) * 512],
                                start=(kt == 0),
                                stop=(kt == 1),
                            )
                    cp(vrep[:, h, ts(c, 1024)], ps[:])

            load_T(qT, q_ld[:1])
            weff_group(0, 1024, 0)
            load_T(featT, f_ld)
            for c in range(2):
                vrep_chunk(c)
            load_T(qT, q_ld[1:])
            weff_group(1024, 1024, 0)
            weff_group(2048, 1024, 0)
            weff_group(R_OFF, 216, 1)
            weff_group(O_OFF, 40, 2)
            for c in range(2, 4):
                vrep_chunk(c)

        # ---- main loop.
        denoms = persist.tile([128, N_QTILES, 2], F32)
        numers = persist.tile([128, N_QTILES], F32)
        with tc.tile_pool(name="mpsum", bufs=2, space="PSUM") as mpsum, \
             tc.tile_pool(name="epool", bufs=4) as epool, \
             tc.tile_pool(name="prodp", bufs=2) as prodp:
            for qt in range(N_QTILES):
                h = 0 if qt < 24 else (1 if qt == 24 else 2)
                e_t = epool.tile([128, PTS], F16, tag="e")
                for half in range(2):
                    ps = mpsum.tile([128, NHALF], F32, tag="mps")
                    for kt in range(2):
                        for c4 in range(NHALF // 512):
                            n0 = half * NHALF + c4 * 512
                            nc.tensor.matmul(
                                ps[:, ts(c4, 512)],
                                weff[:, kt, ts(qt, 128)],
                                featT[:, kt, n0 : n0 + 512],
                                start=(kt == 0),
                                stop=(kt == 1),
                            )
                    nc.scalar.activation(
                        e_t[:, half * NHALF : (half + 1) * NHALF],
                        ps[:],
                        ACTF.Exp,
                        scale=SCALE,
                        accum_out=denoms[:, qt, half : half + 1],
                    )
                prod = prodp.tile([128, PTS], F16, tag="prodv")
                nc.vector.scalar_tensor_tensor(
                    prod[:],
                    e_t[:],
                    1.0,
                    vrep[:, h, :],
                    op0=OP.mult,
                    op1=OP.mult,
                    accum_out=numers[:, qt : qt + 1],
                )

        # ---- epilogue: combine halves, divide, store.
        dsum = persist.tile([128, N_QTILES], F32)
        rcp = persist.tile([128, N_QTILES], F32)
        outt = persist.tile([128, N_QTILES], F32)
        nc.vector.tensor_add(dsum[:], denoms[:, :, 0], denoms[:, :, 1])
        nc.vector.reciprocal(rcp[:], dsum[:])
        nc.vector.tensor_mul(outt[:], numers[:], rcp[:])
        nc.sync.dma_start(res, outt[:])

def _get_nc():
    if "nc" not in _CACHE:
        _CACHE["nc"] = _build_nc()
    return _CACHE["nc"]


def _fast_path_ok(feat, npb, shapes):
    if not all(shapes):
        return False
    if not np.all(npb == PTS):
        return False
    return True


def _score_bound(feat, wks, qs):
    """Cheap upper bound on max |scores| (pre exp). Fast path requires the
    exp outputs to stay well inside fp16 range with no max-subtraction."""
    fn = float(np.sqrt((feat.astype(np.float64) ** 2).sum(1)).max())
    bound = 0.0
    for Wk, q in zip(wks, qs):
        wn = float(np.linalg.norm(Wk.astype(np.float64)))  # ||Wk||_F >= ||Wk||_2
        qn = float(np.sqrt((q.astype(np.float64) ** 2).sum(1)).max())
        bound = max(bound, fn * wn * qn / 16.0)
    return bound


def _fallback(feat, npb, qs, wks, wvs):
    """Exact host computation for inputs outside the device fast path."""
    feat = feat.astype(np.float64)
    ends = np.cumsum(npb)
    starts = ends - npb
    outs = []
    for q, Wk, Wv in zip(qs, wks, wvs):
        q = q.astype(np.float64)
        k = feat @ Wk.astype(np.float64)
        vsum = (feat @ Wv.astype(np.float64)).sum(-1)
        out = np.zeros((B, q.shape[0]), np.float64)
        for b in range(B):
            s, e = int(starts[b]), int(ends[b])
            sc = (k[s:e] @ q.T) / 16.0
            sc -= sc.max(0, keepdims=True)
            ee = np.exp(sc)
            out[b] = (ee * vsum[s:e, None]).sum(0) / ee.sum(0)
        outs.append(out.astype(np.float32))
    return tuple(outs)


def _run(inputs, trace=False):
    feat = np.ascontiguousarray(np.asarray(inputs["feat"], dtype=np.float32))
    npb = np.asarray(inputs["npoints_in_batch"]).astype(np.int64)
    qs = [np.asarray(inputs[f"queries_{n}"], np.float32) for n in "tro"]
    wks = [np.asarray(inputs[f"Wk_{n}"], np.float32) for n in "tro"]
    wvs = [np.asarray(inputs[f"Wv_{n}"], np.float32) for n in "tro"]

    shapes_ok = (
        feat.shape == (B * PTS, D)
        and npb.shape == (B,)
        and qs[0].shape == (QT, D)
        and qs[1].shape == (QR, D)
        and qs[2].shape == (QO, D)
        and all(w.shape == (D, D) for w in wks + wvs)
    )
    if not (shapes_ok and np.all(npb == PTS) and _score_bound(feat, wks, qs) < 9.0):
        return _fallback(feat, npb, qs, wks, wvs), None

    qcat = np.zeros((QPAD, D), np.float32)
    qcat[0:QT] = qs[0]
    qcat[R_OFF : R_OFF + QR] = qs[1]
    qcat[O_OFF : O_OFF + QO] = qs[2]
    assert O_OFF == R_OFF + QR
    wk = np.ascontiguousarray(np.stack(wks))
    wv = np.ascontiguousarray(np.stack(wvs))

    in_maps = [
        {"feat": feat[b * PTS : (b + 1) * PTS], "qcat": qcat, "wk": wk, "wv": wv}
        for b in range(B)
    ]
    r = run_bass_kernel_spmd(_get_nc(), in_maps, core_ids=list(range(8)), trace=trace)
    flat = np.stack([np.asarray(r.results[b]["res"]).T.reshape(-1) for b in range(B)])
    out = (
        flat[:, 0:QT].copy(),
        flat[:, R_OFF : R_OFF + QR].copy(),
        flat[:, O_OFF : O_OFF + QO].copy(),
    )
    return out, r


def kernel(**inputs):
    out, _ = _run(inputs, trace=False)
    return out


def kernel_traced(**inputs):
    """Like kernel() but returns (outputs, BassKernelResults with trace)."""
    return _run(inputs, trace=True)
